# revision 1
# baseline (speedup 1.0000x reference)
"""Single-level 2D Haar DWT (periodization mode) on Trainium2.

Input x: (8, 512, 512, 16) fp32 NHWC. Output: (LL, LH, HL, HH), each
(8, 256, 256, 16) fp32 — +/- combinations of each 2x2 spatial block,
scaled by 0.5.

Sharding: pure data parallel — one batch sample per NeuronCore (8 cores).

Per-core kernel structure (x viewed as (512, 8192) row-major):
  - 2 pair-chunks (128 row-pairs) x 2 W-blocks (half row) = 4 iterations
  - per iter: one 4 MB DMA loads [128 pairs, 2 rows, 4096] into SBUF
    (fully contiguous 16 KB runs per partition),
    8 tensor_add/sub ops implement the 2x2 butterfly (strided SBUF reads),
    4 in-place x0.5 scales, then 4x 1 MB contiguous output DMAs.
"""

import sys

if "/opt/trn_rl_repo" not in sys.path:
    sys.path.insert(0, "/opt/trn_rl_repo")

import numpy as np

B, H, W, C = 8, 512, 512, 16
N_CORES = 8
HO, WO = H // 2, W // 2  # 256, 256
ROW = W * C  # 8192 elements per input row
OROW = WO * C  # 4096 elements per output row

_CACHE = {}


def _build():
    import concourse.bacc as bacc
    import concourse.mybir as mybir
    import concourse.tile as tile

    fp32 = mybir.dt.float32

    nc = bacc.Bacc(
        "TRN2", target_bir_lowering=False, debug=False, num_devices=N_CORES
    )
    x = nc.dram_tensor("x", (H, ROW), fp32, kind="ExternalInput")
    outs = {
        name: nc.dram_tensor(name, (HO, WO, C), fp32, kind="ExternalOutput")
        for name in ("LL", "LH", "HL", "HH")
    }

    # x rows grouped into pairs: [pair q, t in {0,1}, row elems]
    xq = x.rearrange("(q t) m -> q t m", t=2)

    PAIRS = 128  # row-pairs per iteration (partition dim)
    WBLK = ROW // 2  # 4096 input elements per W-block per row
    WQ = WBLK // (2 * C)  # 128 W-pairs per block

    with tile.TileContext(nc) as tc:
        with (
            tc.tile_pool(name="inp", bufs=2) as inp,
            tc.tile_pool(name="mid", bufs=2) as mid,
            tc.tile_pool(name="outp", bufs=2) as outp,
        ):
            for pc in range(H // 2 // PAIRS):  # 2 pair-chunks
                for wb in range(ROW // WBLK):  # 2 W-blocks
                    xt = inp.tile([PAIRS, 2, WBLK], fp32)
                    nc.sync.dma_start(
                        xt[:],
                        xq[
                            pc * PAIRS : (pc + 1) * PAIRS,
                            :,
                            wb * WBLK : (wb + 1) * WBLK,
                        ],
                    )
                    # [pair, t, wq, u, c] view: t = row parity, u = col parity
                    xv = xt[:].rearrange("p t (w u c) -> p t w u c", u=2, c=C)
                    a = xv[:, 0, :, 0, :]
                    b = xv[:, 0, :, 1, :]
                    c_ = xv[:, 1, :, 0, :]
                    d = xv[:, 1, :, 1, :]

                    t1 = mid.tile([PAIRS, WQ, C], fp32, tag="t1")
                    t2 = mid.tile([PAIRS, WQ, C], fp32, tag="t2")
                    u1 = mid.tile([PAIRS, WQ, C], fp32, tag="u1")
                    u2 = mid.tile([PAIRS, WQ, C], fp32, tag="u2")
                    nc.any.tensor_add(t1[:], a, b)
                    nc.any.tensor_add(t2[:], c_, d)
                    nc.any.tensor_sub(u1[:], a, b)
                    nc.any.tensor_sub(u2[:], c_, d)

                    res = {}
                    for name, i0, i1, op in (
                        ("LL", t1, t2, "add"),
                        ("HL", t1, t2, "sub"),
                        ("LH", u1, u2, "add"),
                        ("HH", u1, u2, "sub"),
                    ):
                        ot = outp.tile([PAIRS, WQ, C], fp32, tag=name)
                        if op == "add":
                            nc.any.tensor_add(ot[:], i0[:], i1[:])
                        else:
                            nc.any.tensor_sub(ot[:], i0[:], i1[:])
                        nc.any.tensor_scalar_mul(ot[:], ot[:], 0.5)
                        res[name] = ot

                    for name, ot in res.items():
                        nc.sync.dma_start(
                            outs[name][
                                pc * PAIRS : (pc + 1) * PAIRS,
                                wb * WQ : (wb + 1) * WQ,
                                :,
                            ],
                            ot[:],
                        )

    nc.compile()
    return nc


def _get_nc():
    if "nc" not in _CACHE:
        _CACHE["nc"] = _build()
    return _CACHE["nc"]


def kernel(x):
    from concourse.bass_utils import run_bass_kernel_spmd

    x = np.asarray(x, dtype=np.float32)
    assert x.shape == (B, H, W, C), x.shape

    nc = _get_nc()
    in_maps = [{"x": np.ascontiguousarray(x[i].reshape(H, ROW))} for i in range(B)]
    res = run_bass_kernel_spmd(nc, in_maps, list(range(N_CORES)))

    out = []
    for name in ("LL", "LH", "HL", "HH"):
        out.append(np.stack([res.results[i][name] for i in range(B)], axis=0))
    return tuple(out)
